# revision 6
# baseline (speedup 1.0000x reference)
"""MHSA kernel for 8 Trainium2 NeuronCores.

Distribution (per sharding hint): data-parallel over batch (4) x
tensor-parallel over heads (2 groups of 8 heads) = 8 shards, one per core.

Each core computes, for its (batch b, head-group t):
  qkv projection for its 512 q/k/v dims, attention over its 8 heads,
  and a partial output projection over its 512 v_hat dims.
Host sums the two TP partials per batch and adds the bias.

Runs on the 8 NeuronCores via jax shard_map on the PJRT backend.
"""
import numpy as np
import jax
import jax.numpy as jnp
from jax.sharding import Mesh, PartitionSpec as P
from jax.experimental.shard_map import shard_map
from functools import partial

B, N, C, H = 4, 2048, 1024, 16
HD = C // H  # 64
NCORES = 8
TP = 2              # head groups
HPG = H // TP       # 8 heads per group
DPG = HPG * HD      # 512 dims per group

_compiled = {}


def _shard_fn(x_c, wq_c, wk_c, wv_c, wo_c):
    # per-core shapes (leading core axis of size 1 from shard_map)
    x = x_c[0]        # [N, C]
    wq = wq_c[0]      # [DPG, C]
    wk = wk_c[0]
    wv = wv_c[0]
    wo = wo_c[0]      # [C, DPG]
    scale = HD ** -0.5

    q = x @ wq.T      # [N, DPG]
    k = x @ wk.T
    v = x @ wv.T
    q = q.reshape(N, HPG, HD).transpose(1, 0, 2) * scale   # [HPG, N, HD]
    k = k.reshape(N, HPG, HD).transpose(1, 0, 2)
    v = v.reshape(N, HPG, HD).transpose(1, 0, 2)
    # q-chunked attention: keeps score blocks at [HPG, QB, N] so the
    # softmax works on smaller HBM-resident intermediates per step
    QB = 256
    chunks = []
    for i in range(N // QB):
        qc = q[:, i * QB : (i + 1) * QB]                   # [HPG, QB, HD]
        sc = jnp.einsum("hnc,hmc->hnm", qc, k)             # [HPG, QB, N]
        ac = jax.nn.softmax(sc, axis=-1)
        chunks.append(jnp.einsum("hnm,hmc->hnc", ac, v))   # [HPG, QB, HD]
    vh = jnp.concatenate(chunks, axis=1)                   # [HPG, N, HD]
    vh = vh.transpose(1, 0, 2).reshape(N, DPG)             # [N, DPG]
    out_part = vh @ wo.T                                   # [N, C]
    return out_part[None]


def _get_compiled():
    if "fn" in _compiled:
        return _compiled["fn"], _compiled["mesh"]
    devs = jax.devices()[:NCORES]
    mesh = Mesh(np.asarray(devs), ("core",))
    fn = jax.jit(
        shard_map(
            _shard_fn,
            mesh=mesh,
            in_specs=(P("core"),) * 5,
            out_specs=P("core"),
            check_rep=False,
        )
    )
    _compiled["fn"] = fn
    _compiled["mesh"] = mesh
    return fn, mesh


def _make_shards(x, w_qkv, w_out):
    # per-core input stacks, core c -> (b = c//2, t = c%2)
    w_q = w_qkv[0 * C : 1 * C]          # [C, C]
    w_k = w_qkv[1 * C : 2 * C]
    w_v = w_qkv[2 * C : 3 * C]
    xs, wqs, wks, wvs, wos = [], [], [], [], []
    for c in range(NCORES):
        b, t = c // TP, c % TP
        sl = slice(t * DPG, (t + 1) * DPG)
        xs.append(x[b])
        wqs.append(w_q[sl])
        wks.append(w_k[sl])
        wvs.append(w_v[sl])
        wos.append(w_out[:, sl])
    return (
        np.stack(xs),                   # [8, N, C]
        np.stack(wqs),                  # [8, DPG, C]
        np.stack(wks),
        np.stack(wvs),
        np.stack(wos),                  # [8, C, DPG]
    )


def kernel(x, w_qkv, w_out, b_out):
    x = np.asarray(x, dtype=np.float32)
    w_qkv = np.asarray(w_qkv, dtype=np.float32)
    w_out = np.asarray(w_out, dtype=np.float32)
    b_out = np.asarray(b_out, dtype=np.float32)

    fn, _ = _get_compiled()
    shards = _make_shards(x, w_qkv, w_out)
    parts = np.asarray(jax.block_until_ready(fn(*shards)))   # [8, N, C]

    out = np.empty((B, N, C), dtype=np.float32)
    for b in range(B):
        out[b] = parts[2 * b] + parts[2 * b + 1] + b_out[None, :]
    return out


if __name__ == "__main__":
    rng = np.random.default_rng(0)
    x = rng.standard_normal((B, N, C), dtype=np.float32)
    w_qkv = (rng.standard_normal((3 * C, C), dtype=np.float32) * C ** -0.5)
    w_out = (rng.standard_normal((C, C), dtype=np.float32) * C ** -0.5)
    b_out = rng.standard_normal(C, dtype=np.float32) * 0.01
    o = kernel(x=x, w_qkv=w_qkv, w_out=w_out, b_out=b_out)
    print("kernel ran, out shape", o.shape)
